# revision 40
# baseline (speedup 1.0000x reference)
"""Trainium2 Bass kernel for nn_BlockModel_82678120448388.

Model: per (batch, head): 8x8 transition matrices from an MLP (normalized),
values from a second MLP, then a linear recurrence s_t = A_t s_{t-1} + v_t
over seq=2048.

Sharding: 8 cores = 4 batches x 2 head-halves (32 heads each).

Scan: 16 chunks of 128 tokens in parallel across partitions
(partition = (chunk, head-group)). Phase 1 maintains the [T|u] prefix
per chunk in fp16 (packed innermost layouts -> 2x DVE), storing each
step's prefix (spilled to DRAM). Phase B combines chunk transitions.
Apply phase computes s_r = T_r s0_c + u_r for all r batched (no serial
chain). Emission order: L1(q) | tail(q-1) | L2(q); q3 runs L2 per-tile
(w2 re-streamed) so the last norms/phase1 pipeline instead of bunching.
"""

import numpy as np
import ml_dtypes
from contextlib import ExitStack

import concourse.bass as bass
import concourse.bacc as bacc
import concourse.tile as tile
from concourse import mybir

F32 = mybir.dt.float32
BF16 = mybir.dt.bfloat16
FP16 = mybir.dt.float16
AF = mybir.ActivationFunctionType
ALU = mybir.AluOpType
AX = mybir.AxisListType

BS, SEQ, EMB, BD = 4, 2048, 512, 8
H = EMB // BD
HL = 32            # heads per core
NF = HL * BD * BD  # 2048 blk feats per core
VF = HL * BD       # 256 v feats per core
HID = EMB * BD     # 4096
P = 128
JW = BD + 1        # [T|u] column count (9)
K = 16             # chunks
NHO = P // K       # head-groups per chunk on partitions (8)
NHR = HL // NHO    # heads per group (4)
HRI = NHR * BD     # 32
TUP = NHR * JW * BD  # 288 tu feats per partition
AGF = NHR * BD * BD  # 256 A feats per j-slice
TU8 = 8 * TUP      # 2304 per-tile prefix block
N_CORES = 8


def build_nc(TOK=SEQ):
    C = TOK // K           # tokens per chunk (128)
    QT = min(512, TOK)
    NQ = TOK // QT
    TPQ = QT // P          # token tiles per q (4)
    NT = TOK // P          # total token tiles (16)
    NM = HID // P          # 32 L1 row-tiles
    assert TOK % QT == 0 and QT % P == 0

    nc = bacc.Bacc("TRN2", target_bir_lowering=False, debug=False)

    xT = nc.dram_tensor("xT", [EMB, TOK], BF16, kind="ExternalInput")
    w1 = nc.dram_tensor("w1", [EMB, HID], BF16, kind="ExternalInput")
    b1 = nc.dram_tensor("b1", [HID, 1], F32, kind="ExternalInput")
    w2 = nc.dram_tensor("w2", [HID, NF], BF16, kind="ExternalInput")
    b2 = nc.dram_tensor("b2", [1, NF], BF16, kind="ExternalInput")
    v1 = nc.dram_tensor("v1", [EMB, EMB], BF16, kind="ExternalInput")
    c1 = nc.dram_tensor("c1", [EMB, 1], F32, kind="ExternalInput")
    v2 = nc.dram_tensor("v2", [EMB, VF], BF16, kind="ExternalInput")
    c2 = nc.dram_tensor("c2", [1, VF], BF16, kind="ExternalInput")
    a0 = nc.dram_tensor("a0", [NHO, HRI], FP16, kind="ExternalInput")
    out = nc.dram_tensor("out", [TOK, VF], F32, kind="ExternalOutput")

    # scratch row (tau*128 + j*16 + c) holds token c*C + 8*tau + j
    a_dram = nc.dram_tensor("a_scratch", [TOK, NF], FP16)
    v_dram = nc.dram_tensor("v_scratch", [TOK, VF], FP16)
    tu_dram = nc.dram_tensor("tu_scratch", [NT * P, TU8], FP16)

    with ExitStack() as ctx:
        tc = ctx.enter_context(tile.TileContext(nc))
        cpool = ctx.enter_context(tc.tile_pool(name="consts", bufs=1))
        wpool = ctx.enter_context(tc.tile_pool(name="weights", bufs=1))
        xpool = ctx.enter_context(tc.tile_pool(name="xstream", bufs=2))
        w1pool = ctx.enter_context(tc.tile_pool(name="w1s", bufs=2))
        hpool = ctx.enter_context(tc.tile_pool(name="hidden", bufs=1))
        w2pool = ctx.enter_context(tc.tile_pool(name="w2s", bufs=2))
        pspool = ctx.enter_context(tc.tile_pool(name="ps", bufs=8, space="PSUM"))
        blkpool = ctx.enter_context(tc.tile_pool(name="blk", bufs=2 * TPQ))
        sqpool = ctx.enter_context(tc.tile_pool(name="sq", bufs=3))
        smpool = ctx.enter_context(tc.tile_pool(name="small", bufs=3))
        apool = ctx.enter_context(tc.tile_pool(name="aT", bufs=2))
        vtpool = ctx.enter_context(tc.tile_pool(name="vT", bufs=1))
        agpool = ctx.enter_context(tc.tile_pool(name="ag", bufs=2))
        vgpool = ctx.enter_context(tc.tile_pool(name="vg", bufs=NT))
        tupool = ctx.enter_context(tc.tile_pool(name="tu", bufs=3))
        turpool = ctx.enter_context(tc.tile_pool(name="tur", bufs=2))
        mopool = ctx.enter_context(tc.tile_pool(name="mo", bufs=1))
        s1pool = ctx.enter_context(tc.tile_pool(name="s1", bufs=1))
        s2pool = ctx.enter_context(tc.tile_pool(name="s2", bufs=1))
        scpool = ctx.enter_context(tc.tile_pool(name="scan", bufs=1))
        pcpool = ctx.enter_context(tc.tile_pool(name="pc", bufs=2))

        ctx.enter_context(nc.allow_low_precision(reason="fp16 scan state"))

        # ---- prefetch window-0 activations before the bulky consts ----
        xq0 = xpool.tile([P, 4, 4 * P], BF16, tag="xq", name="xq0")
        for k in range(4):
            nc.sync.dma_start(
                xq0[:, k, :], bass.AP(xT, k * P * TOK, [[TOK, P], [1, 4 * P]]))

        # ---- constants / weights ----
        ones_s = cpool.tile([1, P], BF16, tag="ones")
        nc.vector.memset(ones_s[:], 1.0)
        b1_s = cpool.tile([P, NM], F32, tag="b1")
        nc.sync.dma_start(b1_s[:], b1[:].rearrange("(m p) one -> p (m one)", p=P))
        c1_s = cpool.tile([P, EMB // P], F32, tag="c1")
        nc.sync.dma_start(c1_s[:], c1[:].rearrange("(m p) one -> p (m one)", p=P))
        b2_s = cpool.tile([1, NF], BF16, tag="b2")
        nc.sync.dma_start(b2_s[:], b2[:])
        c2_s = cpool.tile([1, VF], BF16, tag="c2")
        nc.sync.dma_start(c2_s[:], c2[:])
        a0_s = cpool.tile([NHO, HRI], FP16, tag="a0")
        nc.sync.dma_start(a0_s[:], a0[:])

        v1_s = wpool.tile([P, 4, EMB], BF16, tag="v1")
        nc.sync.dma_start(v1_s[:], v1[:].rearrange("(k p) m -> p k m", p=P))
        v2_s = wpool.tile([P, 4, VF], BF16, tag="v2")
        nc.sync.dma_start(v2_s[:], v2[:].rearrange("(k p) n -> p k n", p=P))

        vg_tiles = [None] * NT
        hid_box = {}
        blk_box = {}
        st = {'tu8': None, 'prev_tu8': None}

        # ================= phase 1 =================
        # tu slot layout per partition (c,ho): feat hr*72 + j*8 + k =
        # [T|u][row k, col j]. ag: (j, hr, i, k) A row-major. vg: (j, hr, i).
        def phase1_tile(tau):
            ag, vg = st['ag'], vg_tiles[tau]
            tu8 = tupool.tile([P, TU8], FP16, tag="tu8", name=f"tu8_{tau}")
            for jr in range(8):
                r = tau * 8 + jr
                dst = jr * TUP

                def dap(off, dims):
                    return bass.AP(tu8.tensor, tu8[:].offset + dst + off,
                                   [[TU8, P]] + dims)

                if r == 0:
                    # Tu[k,j] = A[k,j]; u = v_0
                    nc.vector.tensor_copy(
                        dap(0, [[BD * JW, NHR], [BD, BD], [1, BD]]),
                        bass.AP(ag.tensor, ag[:].offset + jr * AGF,
                                [[8 * AGF, P], [BD * BD, NHR], [1, BD], [BD, BD]]))
                    nc.vector.tensor_copy(
                        dap(BD * BD, [[BD * JW, NHR], [1, BD]]),
                        bass.AP(vg.tensor, vg[:].offset + jr * HRI,
                                [[8 * HRI, P], [BD, NHR], [1, BD]]))
                    continue
                if jr == 0:
                    src_t, src_off = st['prev_tu8'], 7 * TUP
                else:
                    src_t, src_off = tu8, (jr - 1) * TUP

                def sap(off, dims):
                    return bass.AP(src_t.tensor, src_t[:].offset + src_off + off,
                                   [[TU8, P]] + dims)

                agb = ag[:].offset + jr * AGF
                # mo[hr][j, i, k] = A[i, k] * Tu[k, j]
                mo = mopool.tile([P, TUP * BD], FP16, tag="mo", name=f"mo{r}")
                MOF = TUP * BD
                for hr in range(NHR):
                    nc.vector.tensor_tensor(
                        bass.AP(mo.tensor, mo[:].offset + hr * JW * BD * BD,
                                [[MOF, P], [BD * BD, JW], [BD, BD], [1, BD]]),
                        bass.AP(ag.tensor, agb + hr * BD * BD,
                                [[8 * AGF, P], [0, JW], [BD, BD], [1, BD]]),
                        sap(hr * JW * BD, [[BD, JW], [0, BD], [1, BD]]),
                        ALU.mult)
                # sum over k (3 adds), then u += v
                s1 = s1pool.tile([P, TUP * 4], FP16, tag="s1", name=f"s1_{r}")
                S1F = TUP * 4
                nc.vector.tensor_tensor(
                    bass.AP(s1.tensor, s1[:].offset,
                            [[S1F, P], [32, 36], [4, BD], [1, 4]]),
                    bass.AP(mo.tensor, mo[:].offset,
                            [[MOF, P], [64, 36], [8, BD], [1, 4]]),
                    bass.AP(mo.tensor, mo[:].offset + 4,
                            [[MOF, P], [64, 36], [8, BD], [1, 4]]),
                    ALU.add)
                s2 = s2pool.tile([P, TUP * 2], FP16, tag="s2", name=f"s2_{r}")
                S2F = TUP * 2
                nc.vector.tensor_tensor(
                    bass.AP(s2.tensor, s2[:].offset,
                            [[S2F, P], [16, 36], [2, BD], [1, 2]]),
                    bass.AP(s1.tensor, s1[:].offset,
                            [[S1F, P], [32, 36], [4, BD], [1, 2]]),
                    bass.AP(s1.tensor, s1[:].offset + 2,
                            [[S1F, P], [32, 36], [4, BD], [1, 2]]),
                    ALU.add)
                nc.vector.tensor_tensor(
                    dap(0, [[BD, 36], [1, BD]]),
                    bass.AP(s2.tensor, s2[:].offset, [[S2F, P], [16, 36], [2, BD]]),
                    bass.AP(s2.tensor, s2[:].offset + 1, [[S2F, P], [16, 36], [2, BD]]),
                    ALU.add)
                nc.vector.tensor_tensor(
                    dap(BD * BD, [[JW * BD, NHR], [1, BD]]),
                    dap(BD * BD, [[JW * BD, NHR], [1, BD]]),
                    bass.AP(vg.tensor, vg[:].offset + jr * HRI,
                            [[8 * HRI, P], [BD, NHR], [1, BD]]),
                    ALU.add)
            st['dma'].dma_start(tu_dram[bass.ds(tau * P, P), :], tu8[:])
            st['prev_tu8'] = tu8

        # ================= stage A sections =================
        # Uneven windows: the last windows are small so the final phase-1
        # chain segments (which can only run after their window's L2) are
        # short, shrinking the serial tail. w2 is re-streamed per window.
        WSIZES = [4, 4, 4, 4]
        WT0 = [sum(WSIZES[:i]) for i in range(len(WSIZES))]
        NW = len(WSIZES)
        assert sum(WSIZES) == NT

        def emit_L1V(w):
            tau0, nt = WT0[w], WSIZES[w]
            QTw = nt * P
            if w == 0:
                xq = xq0
            else:
                xq = xpool.tile([P, 4, QTw], BF16, tag="xq", name=f"xq{w}")
                for k in range(4):
                    nc.sync.dma_start(
                        xq[:, k, :],
                        bass.AP(xT, k * P * TOK + tau0 * P, [[TOK, P], [1, QTw]]))
            # v path first: psv matmuls can start as soon as hv relus land
            hv_t = hpool.tile([P, 4, QTw], BF16, tag="hv", name=f"hv{w}")
            for m in range(4):
                ps = pspool.tile([P, 512], F32, tag="ps")
                for k in range(4):
                    nc.tensor.matmul(ps[:, 0:QTw], v1_s[:, k, bass.ts(m, P)],
                                     xq[:, k, :], start=(k == 0), stop=(k == 3))
                nc.scalar.activation(hv_t[:, m, :], ps[:, 0:QTw], AF.Relu,
                                     bias=c1_s[:, m:m + 1])
            for ttq in range(nt):
                tau = tau0 + ttq
                psvt = pspool.tile([P, 512], F32, tag="ps")
                psv = psvt[:, 0:VF]
                nc.tensor.matmul(psv, ones_s[:1, :], c2_s[:1, :],
                                 start=True, stop=False)
                for k in range(4):
                    nc.tensor.matmul(psv, hv_t[:, k, bass.ts(ttq, P)],
                                     v2_s[:, k, :], start=False, stop=(k == 3))
                vt = vtpool.tile([P, VF], FP16, tag="vt")
                nc.scalar.activation(vt[:], psv, AF.Identity)
                nc.scalar.dma_start(
                    bass.AP(v_dram, tau * P * VF,
                            [[VF, K], [K * VF, 8], [1, VF]]),
                    vt[:])
                vg = vgpool.tile([P, 8 * HRI], FP16, tag="vg", name=f"vg{tau}")
                nc.scalar.dma_start(vg[:], bass.AP(
                    v_dram, tau * P * VF,
                    [[HRI, P], [K * VF, 8], [1, HRI]]))
                vg_tiles[tau] = vg
            hid_t = hpool.tile([P, NM, QTw], BF16, tag="hid", name=f"hid{w}")
            for mb in range(4):
                w1b = w1pool.tile([P, 4, 8 * P], BF16, tag="w1b",
                                  name=f"w1b{w}_{mb}")
                nc.sync.dma_start(
                    w1b[:], w1[:, bass.ts(mb, 8 * P)]
                    .rearrange("(k p) m -> p k m", p=P))
                for m8 in range(8):
                    m = mb * 8 + m8
                    ps = pspool.tile([P, 512], F32, tag="ps")
                    for k in range(4):
                        nc.tensor.matmul(ps[:, 0:QTw], w1b[:, k, bass.ts(m8, P)],
                                         xq[:, k, :], start=(k == 0), stop=(k == 3))
                    nc.scalar.activation(hid_t[:, m, :], ps[:, 0:QTw], AF.Relu,
                                         bias=b1_s[:, m:m + 1])
            hid_box[w] = (hid_t, hv_t)

        tail_state = {}

        def norm_tile(w, ttq, pw):
            """DVE reduce / max; returns dm (f32) for rc computation."""
            pst = smpool.tile([P, HL * BD], FP16, tag="pst",
                              name=f"pst{w}_{ttq}")
            nc.vector.tensor_reduce(
                bass.AP(pst.tensor, pst[:].offset,
                        [[HL * BD, P], [BD, HL], [1, BD]]),
                bass.AP(pw.tensor, pw[:].offset,
                        [[NF, P], [64, HL], [1, BD], [BD, BD]]),
                axis=AX.X, op=ALU.add)
            dm = smpool.tile([P, HL], F32, tag="dm", name=f"dm{w}_{ttq}")
            nc.vector.tensor_reduce(
                dm[:].rearrange("p (h one) -> p h one", h=HL, one=1),
                pst[:].rearrange("p (h k) -> p h k", h=HL, k=BD),
                axis=AX.X, op=ALU.max)
            return dm

        def scan_tile(w, ttq, rc, dmae):
            """A = blk * rc; scratch round trip; phase 1 steps."""
            tau = WT0[w] + ttq
            rcr = smpool.tile([P, HL * BD], FP16, tag="rcr")
            nc.vector.tensor_copy(
                bass.AP(rcr.tensor, rcr[:].offset,
                        [[HL * BD, P], [BD, HL], [1, BD]]),
                bass.AP(rc.tensor, rc[:].offset, [[HL, P], [1, HL], [0, BD]]))
            aT = apool.tile([P, NF], FP16, tag="aT")
            nc.vector.tensor_tensor(
                bass.AP(aT.tensor, aT[:].offset,
                        [[NF, P], [64, HL], [BD, BD], [1, BD]]),
                bass.AP(blk_box[w][ttq].tensor, blk_box[w][ttq][:].offset,
                        [[NF, P], [64, HL], [BD, BD], [1, BD]]),
                bass.AP(rcr.tensor, rcr[:].offset,
                        [[HL * BD, P], [BD, HL], [0, BD], [1, BD]]),
                ALU.mult)
            dmae.dma_start(
                bass.AP(a_dram, tau * P * NF,
                        [[NF, K], [K * NF, 8], [1, NF]]),
                aT[:])
            ag = agpool.tile([P, 8 * AGF], FP16, tag="ag", name=f"ag{tau}")
            dmae.dma_start(ag[:], bass.AP(
                a_dram, tau * P * NF,
                [[AGF, P], [K * NF, 8], [1, AGF]]))
            st['ag'] = ag
            phase1_tile(tau)

        def emit_tail_stage(w, stage):
            """Norm stages for w's tiles, batched per act function."""
            nt = WSIZES[w]
            if stage == 0:
                pws = []
                for ttq in range(nt):
                    pw = sqpool.tile([P, NF], FP16, tag="pw", name=f"pw{w}_{ttq}")
                    nc.scalar.activation(pw[:], blk_box[w][ttq][:], AF.Square)
                    pws.append(pw)
                tail_state[w] = {'pws': pws}
            elif stage == 1:
                # pw = exp(0.6*ln(blk^2)) = |blk|^1.2
                for pw in tail_state[w]['pws']:
                    nc.scalar.activation(pw[:], pw[:], AF.Ln)
                for pw in tail_state[w]['pws']:
                    nc.scalar.activation(pw[:], pw[:], AF.Exp, scale=0.6)
            elif stage == 2:
                tail_state[w]['dms'] = [
                    norm_tile(w, ttq, tail_state[w]['pws'][ttq])
                    for ttq in range(nt)]
            elif stage == 3:
                rcs = []
                for ttq in range(nt):
                    rc = smpool.tile([P, HL], FP16, tag="rc", name=f"rc{w}_{ttq}")
                    nc.scalar.activation(rc[:], tail_state[w]['dms'][ttq][:], AF.Ln)
                    rcs.append(rc)
                for rc in rcs:
                    nc.scalar.activation(rc[:], rc[:], AF.Exp, scale=-1.0 / 1.2)
                tail_state[w]['rcs'] = rcs

        def emit_tail_scan(w):
            for ttq in range(WSIZES[w]):
                scan_tile(w, ttq, tail_state[w]['rcs'][ttq], nc.scalar)

        def emit_tail_final(w, ttqs):
            """Last window: per-tile norm+scan, rc acts of the previous
            tile ride the next tile's Ln/Exp batches (fewer table loads)."""
            pend = None  # (ttq, dm)
            for ttq in ttqs:
                pw = sqpool.tile([P, NF], FP16, tag="pw", name=f"pwF_{ttq}")
                nc.scalar.activation(pw[:], blk_box[w][ttq][:], AF.Square)
                nc.scalar.activation(pw[:], pw[:], AF.Ln)
                if pend is not None:
                    rcp = smpool.tile([P, HL], FP16, tag="rc",
                                      name=f"rcF_{pend[0]}")
                    nc.scalar.activation(rcp[:], pend[1][:], AF.Ln)
                nc.scalar.activation(pw[:], pw[:], AF.Exp, scale=0.6)
                if pend is not None:
                    nc.scalar.activation(rcp[:], rcp[:], AF.Exp, scale=-1.0 / 1.2)
                    scan_tile(w, pend[0], rcp, nc.sync)
                dm = norm_tile(w, ttq, pw)
                pend = (ttq, dm)
            rc = smpool.tile([P, HL], FP16, tag="rc", name=f"rcF_{pend[0]}")
            nc.scalar.activation(rc[:], pend[1][:], AF.Ln)
            nc.scalar.activation(rc[:], rc[:], AF.Exp, scale=-1.0 / 1.2)
            scan_tile(w, pend[0], rc, nc.sync)

        def emit_L2(w, ttqs, tail_w):
            hid_t, _ = hid_box[w]
            blks = blk_box.setdefault(w, {})
            for i in ttqs:
                blks[i] = blkpool.tile([P, NF], FP16, tag="blk",
                                       name=f"blk{w}_{i}")
            for n in range(NF // 512):
                pss = {i: pspool.tile([P, 512], F32, tag="ps",
                                      name=f"l2ps{w}_{n}_{i}")
                       for i in ttqs}
                for ttq in ttqs:
                    nc.tensor.matmul(pss[ttq][:], ones_s[:1, :],
                                     b2_s[:1, bass.ts(n, 512)], start=True, stop=False)
                for kg in range(NM // 4):
                    w2t = w2pool.tile([P, 4, 512], BF16, tag="w2t")
                    nc.sync.dma_start(
                        w2t[:], bass.AP(w2, (kg * 4 * P) * NF + n * 512,
                                        [[NF, P], [P * NF, 4], [1, 512]]))
                    for k4 in range(4):
                        k = kg * 4 + k4
                        for ttq in ttqs:
                            nc.tensor.matmul(pss[ttq][:], hid_t[:, k, bass.ts(ttq, P)],
                                             w2t[:, k4, :], start=False,
                                             stop=(k == NM - 1))
                for ttq in ttqs:
                    nc.scalar.activation(blks[ttq][:, bass.ts(n, 512)], pss[ttq][:],
                                         AF.Identity)
                if tail_w is not None:
                    emit_tail_stage(tail_w, n)

        # ================= emit: L1(w) | L2(w) x tail(w-1) =================
        # Last window: two 2-tile L2 passes (w2 streamed twice) so its blks
        # complete staggered and the final phase-1 segments pipeline.
        st['dma'] = nc.scalar
        for w in range(NW - 1):
            emit_L1V(w)
            emit_L2(w, range(TPQ), w - 1 if w > 0 else None)
            if w > 0:
                emit_tail_scan(w - 1)
        wl = NW - 1
        emit_L1V(wl)
        emit_L2(wl, range(TPQ), wl - 1)
        emit_tail_scan(wl - 1)
        st['dma'] = nc.sync
        emit_tail_final(wl, range(TPQ))

        # ================= phase B: combine chunk transitions =================
        tu_last = st['prev_tu8']
        LOFF = 7 * TUP
        tuB = scpool.tile([NHO, K * TUP], FP16, tag="tuB")
        for c in range(K):
            nc.sync.dma_start(tuB[:, c * TUP:(c + 1) * TUP],
                              tu_last[c * NHO:(c + 1) * NHO,
                                      LOFF:LOFF + TUP])
        s_seq = scpool.tile([NHO, (K + 1) * HRI], FP16, tag="sseq")
        nc.vector.tensor_copy(s_seq[:, 0:HRI], a0_s[:])
        SSF = (K + 1) * HRI
        TBF = K * TUP
        for c in range(K):
            moB = pcpool.tile([NHO, AGF], FP16, tag="moB", name=f"moB{c}")
            nc.vector.tensor_tensor(
                bass.AP(moB.tensor, moB[:].offset,
                        [[AGF, NHO], [64, NHR], [8, BD], [1, BD]]),
                bass.AP(tuB.tensor, tuB[:].offset + c * TUP,
                        [[TBF, NHO], [JW * BD, NHR], [BD, BD], [1, BD]]),
                bass.AP(s_seq.tensor, s_seq[:].offset + c * HRI,
                        [[SSF, NHO], [BD, NHR], [1, BD], [0, BD]]),
                ALU.mult)
            sred = smpool.tile([NHO, HRI], FP16, tag="sred", name=f"sred{c}")
            nc.vector.tensor_reduce(
                bass.AP(sred.tensor, sred[:].offset, [[HRI, NHO], [BD, NHR], [1, BD]]),
                bass.AP(moB.tensor, moB[:].offset,
                        [[AGF, NHO], [64, NHR], [1, BD], [8, BD]]),
                axis=AX.X, op=ALU.add)
            nc.vector.tensor_tensor(
                bass.AP(s_seq.tensor, s_seq[:].offset + (c + 1) * HRI,
                        [[SSF, NHO], [BD, NHR], [1, BD]]),
                bass.AP(sred.tensor, sred[:].offset, [[HRI, NHO], [BD, NHR], [1, BD]]),
                bass.AP(tuB.tensor, tuB[:].offset + c * TUP + BD * BD,
                        [[TBF, NHO], [JW * BD, NHR], [1, BD]]),
                ALU.add)
        s_init = scpool.tile([P, HRI], FP16, tag="sinit")
        for c in range(K):
            nc.sync.dma_start(s_init[c * NHO:(c + 1) * NHO, :],
                              s_seq[:, c * HRI:(c + 1) * HRI])
        # srepA[hr*64 + k*8 + i] = s0[hr, k], bcast over i
        srepA = scpool.tile([P, AGF], FP16, tag="srepA")
        nc.vector.tensor_copy(
            bass.AP(srepA.tensor, srepA[:].offset,
                    [[AGF, P], [64, NHR], [8, BD], [1, BD]]),
            bass.AP(s_init.tensor, s_init[:].offset,
                    [[HRI, P], [BD, NHR], [1, BD], [0, BD]]))

        # ================= apply: s_r = T_r s0 + u_r (batched) =================
        for tau in range(NT):
            tu8 = turpool.tile([P, TU8], FP16, tag="tu8r", name=f"tu8r_{tau}")
            nc.sync.dma_start(tu8[:], tu_dram[bass.ds(tau * P, P), :])
            # mof[hr][jr, k, i] = T_r[i,k] * s0[k]   (T[i,k] at j=k col, row i)
            mof = mopool.tile([P, 2048], FP16, tag="mof", name=f"mof{tau}")
            for hr in range(NHR):
                nc.vector.tensor_tensor(
                    bass.AP(mof.tensor, mof[:].offset + hr * 512,
                            [[2048, P], [64, 8], [8, BD], [1, BD]]),
                    bass.AP(tu8.tensor, tu8[:].offset + hr * JW * BD,
                            [[TU8, P], [TUP, 8], [8, BD], [1, BD]]),
                    bass.AP(srepA.tensor, srepA[:].offset + hr * 64,
                            [[AGF, P], [0, 8], [8, BD], [1, BD]]),
                    ALU.mult)
            # sum over k: (hr.jr merged: stride 64, count 32)
            s1a = s1pool.tile([P, 1024], FP16, tag="s1a", name=f"s1a{tau}")
            nc.vector.tensor_tensor(
                bass.AP(s1a.tensor, s1a[:].offset, [[1024, P], [32, 32], [8, 4], [1, BD]]),
                bass.AP(mof.tensor, mof[:].offset, [[2048, P], [64, 32], [8, 4], [1, BD]]),
                bass.AP(mof.tensor, mof[:].offset + 32,
                        [[2048, P], [64, 32], [8, 4], [1, BD]]),
                ALU.add)
            s2a = s2pool.tile([P, 512], FP16, tag="s2a", name=f"s2a{tau}")
            nc.vector.tensor_tensor(
                bass.AP(s2a.tensor, s2a[:].offset, [[512, P], [16, 32], [8, 2], [1, BD]]),
                bass.AP(s1a.tensor, s1a[:].offset, [[1024, P], [32, 32], [16, 2], [1, BD]]),
                bass.AP(s1a.tensor, s1a[:].offset + 8,
                        [[1024, P], [32, 32], [16, 2], [1, BD]]),
                ALU.add)
            s3 = s2pool.tile([P, 256], FP16, tag="s3", name=f"s3_{tau}")
            nc.vector.tensor_tensor(
                bass.AP(s3.tensor, s3[:].offset, [[256, P], [8, 32], [1, BD]]),
                bass.AP(s2a.tensor, s2a[:].offset, [[512, P], [16, 32], [1, BD]]),
                bass.AP(s2a.tensor, s2a[:].offset + 8, [[512, P], [16, 32], [1, BD]]),
                ALU.add)
            # st[jr, hr, i] = s3[hr, jr, i] + u[jr, hr, i]
            stt = pcpool.tile([P, 256], FP16, tag="stt", name=f"stt{tau}")
            nc.vector.tensor_tensor(
                bass.AP(stt.tensor, stt[:].offset,
                        [[256, P], [HRI, 8], [BD, NHR], [1, BD]]),
                bass.AP(s3.tensor, s3[:].offset,
                        [[256, P], [BD, 8], [64, NHR], [1, BD]]),
                bass.AP(tu8.tensor, tu8[:].offset + BD * BD,
                        [[TU8, P], [TUP, 8], [JW * BD, NHR], [1, BD]]),
                ALU.add)
            s32 = pcpool.tile([P, 256], F32, tag="s32", name=f"s32_{tau}")
            nc.scalar.activation(s32[:], stt[:], AF.Identity)
            nc.sync.dma_start(out[bass.ds(tau * P, P), :], s32[:])

    nc.compile()
    return nc


# ---------------- host side ----------------

_NC_CACHE = {}


def _get_nc(TOK=SEQ):
    if TOK not in _NC_CACHE:
        _NC_CACHE[TOK] = build_nc(TOK=TOK)
    return _NC_CACHE[TOK]


def prep_shared(W1, b1, W2, b2, V1, c1, V2, c2, a0):
    bf = ml_dtypes.bfloat16
    W2r = W2.reshape(H, BD, BD, HID)
    W2c = (W2r - W2r.mean(axis=1, keepdims=True)).reshape(H * BD * BD, HID)
    b2r = b2.reshape(H, BD, BD)
    b2c = (b2r - b2r.mean(axis=1, keepdims=True)).reshape(-1)
    shared = {
        "w1": np.ascontiguousarray(W1.T).astype(bf),
        "b1": np.asarray(b1).reshape(HID, 1).astype(np.float32),
        "v1": np.ascontiguousarray(V1.T).astype(bf),
        "c1": np.asarray(c1).reshape(EMB, 1).astype(np.float32),
    }
    halves = []
    for half in range(2):
        rsl = slice(half * NF, (half + 1) * NF)
        vsl = slice(half * VF, (half + 1) * VF)
        hsl = slice(half * HL, (half + 1) * HL)
        a0h = np.asarray(a0)[0, hsl]                       # [32, 8]
        a0p = a0h.reshape(NHO, NHR * BD)                   # [ho, (hr, i)]
        halves.append({
            "w2": np.ascontiguousarray(W2c[rsl].T).astype(bf),
            "b2": b2c[rsl].reshape(1, NF).astype(bf),
            "v2": np.ascontiguousarray(V2[vsl].T).astype(bf),
            "c2": np.asarray(c2)[vsl].reshape(1, VF).astype(bf),
            "a0": a0p.astype(np.float16),
        })
    return shared, halves


def make_in_maps(x, W1, b1, W2, b2, V1, c1, V2, c2, a0):
    shared, halves = prep_shared(W1, b1, W2, b2, V1, c1, V2, c2, a0)
    bf = ml_dtypes.bfloat16
    in_maps = []
    for core in range(N_CORES):
        b, half = core // 2, core % 2
        m = dict(shared)
        m.update(halves[half])
        # column order (tau, c, j): col = tau*128 + c*8 + j for
        # token t = c*128 + tau*8 + j
        xs = np.asarray(x)[b].T.reshape(EMB, K, SEQ // P, 8)
        m["xT"] = np.ascontiguousarray(
            xs.transpose(0, 2, 1, 3).reshape(EMB, SEQ)).astype(bf)
        in_maps.append(m)
    return in_maps


def kernel(x, W1, b1, W2, b2, V1, c1, V2, c2, a0):
    from concourse import bass_utils
    nc = _get_nc(SEQ)
    in_maps = make_in_maps(x, W1, b1, W2, b2, V1, c1, V2, c2, a0)
    res = bass_utils.run_bass_kernel_spmd(nc, in_maps, core_ids=list(range(N_CORES)))
    out = np.zeros((BS, SEQ, EMB), np.float32)
    for core in range(N_CORES):
        b, half = core // 2, core % 2
        # kernel row = tau*128 + c*8 + ho, col = jr*32 + hr*8 + i
        # true row t = c*128 + tau*8 + jr, col = ho*32 + hr*8 + i
        r = res.results[core]["out"].reshape(SEQ // P, K, NHO, 8, NHR, BD)
        r = r.transpose(1, 0, 3, 2, 4, 5).reshape(SEQ, VF)
        out[b, :, half * VF:(half + 1) * VF] = r
    return out
